# revision 40
# baseline (speedup 1.0000x reference)
"""GAT (2-layer graph attention network) Trainium2 Bass kernel, 8-core SPMD.

Sharding: core c computes head c of layer 1 (head-parallel) and rows
[c*512,(c+1)*512) of the single-head output layer (row-parallel), with a
ReduceScatter+AllGather exchange of the per-head h @ Wo partial products.

Key identity: exp(leaky_relu(s, a)) = max(exp(s), exp(a*s)) for 0<a<=1, and
s = f_src[i] + f_dst[j] makes each exp an outer product. The per-j factor
exp(f_dst[j]) is folded into the matmul weights (including the appended
ones-column that produces the softmax denominator), so the N^2 inner loop is
one ACT scale pass, one DVE max, one DVE mask-multiply, and the PE matmul.
Constant shifts C_SRC/C_DST keep fp16 intermediates in range; they cancel in
the softmax normalization exactly.

kernel(**inputs) takes full unsharded inputs, returns the full output.
"""

from contextlib import ExitStack

import numpy as np

import concourse.mybir as mybir
import concourse.tile as tile
from concourse import bacc
from concourse.bass_utils import run_bass_kernel_spmd
from concourse.masks import make_identity

# Steer every activation to the one ACT table set covering all functions this
# kernel uses (Exp, Identity, Ln) so no mid-kernel table reloads are needed.
# Set IDs are positions in act_info.json's list, so keep the dict order and
# blank out the other sets rather than filtering them.
_orig_get_tables = bacc.get_activation_tables


def _pinned_tables(arch):
    tabs = _orig_get_tables(arch)
    if "natural_log_exp_and_others" in tabs:
        return {name: (funcs if name == "natural_log_exp_and_others" else set())
                for name, funcs in tabs.items()}
    return tabs


bacc.get_activation_tables = _pinned_tables

N = 4096
F = 512
D = 64          # per-head hidden == n classes
H = 8
ALPHA = 0.2
N_CORES = 8
P = 128
NT = N // P             # 32 node tiles
SLICE = N // N_CORES    # 512 rows per core in layer 2
NKF = F // P            # 4 contraction tiles for x @ W
C_SRC = 2.0             # constant exp shifts (cancel in softmax)
C_DST = 1.0
SPLIT_A = 2048          # layer-1 attention piece-A columns (piece B = N - SPLIT_A)
RS_A = SPLIT_A // N_CORES
RS_B = (N - SPLIT_A) // N_CORES

F32 = mybir.dt.float32
F16 = mybir.dt.float16

_CACHED = {}


def _weight_prep(nc, pools, tag, dt_fast, src_fn, extra_fn=None,
                 t_order=None):
    """Per node tile t, src_fn(t) emits + returns a [P, D+2] staging AP
    (Wh cols | f_src | f_dst). Builds scaled lhsT tiles
    whs = exp(f_dst - C_DST) * [Wh | 1] and r = exp(-(1-ALPHA) f_dst)."""
    persist, small = pools["persist"], pools["small"]
    whs_tiles = {}
    r_sb = persist.tile([P, NT], F32, tag=f"r_{tag}", name=f"r_{tag}")
    for t in (t_order if t_order is not None else range(NT)):
        src = src_fn(t)
        whs = persist.tile([P, D + 1], dt_fast, tag=f"whs_{tag}_{t}",
                           name=f"whs_{tag}_{t}")
        e1 = small.tile([P, 1], F32, tag="e1", bufs=4, name=f"e1_{tag}_{t}")
        nc.scalar.activation(e1[:], src[:, D + 1:D + 2],
                             mybir.ActivationFunctionType.Exp,
                             bias=_CACHED["bias_d"][:])
        nc.scalar.activation(r_sb[:, t:t + 1], src[:, D + 1:D + 2],
                             mybir.ActivationFunctionType.Exp,
                             scale=-(1.0 - ALPHA))
        nc.vector.tensor_scalar_mul(whs[:, 0:D], src[:, 0:D], e1[:])
        nc.vector.tensor_copy(whs[:, D:D + 1], e1[:])
        if extra_fn is not None:
            extra_fn(t, src)
        whs_tiles[t] = whs
    return whs_tiles, r_sb


def _bcast_exp_rows(nc, pools, tag, dt_fast, row_ap, width):
    """From row_ap [1, width] (raw f_src on partition 0), build [P, width]
    tiles F1 = exp(f_src - C_SRC), F2 = exp(ALPHA*f_src - C_SRC)."""
    persist, psum = pools["persist"], pools["psum"]
    ones1 = _CACHED["ones16"] if row_ap.dtype == F16 else _CACHED["ones1"]
    f1 = persist.tile([P, width], dt_fast, tag=f"f1_{tag}", name=f"f1_{tag}")
    f2 = persist.tile([P, width], dt_fast, tag=f"f2_{tag}", name=f"f2_{tag}")
    for c in range(width // 512):
        bc_ps = psum.tile([P, 512], F32, tag="bank", bufs=8,
                          name=f"bc_{tag}_{c}")
        nc.tensor.matmul(bc_ps[:], ones1[0:1, :],
                         row_ap[0:1, c * 512:(c + 1) * 512],
                         start=True, stop=True)
        nc.scalar.activation(f1[:, c * 512:(c + 1) * 512], bc_ps[:],
                             mybir.ActivationFunctionType.Exp,
                             bias=_CACHED["bias_s"][:])
        nc.scalar.activation(f2[:, c * 512:(c + 1) * 512], bc_ps[:],
                             mybir.ActivationFunctionType.Exp,
                             bias=_CACHED["bias_s"][:], scale=ALPHA)
    return f1, f2


CCE_MUL = False


def _attention(nc, pools, tag, dt_fast, whs_tiles, r_sb, f1_bc, f2_bc,
               adjt_ap, out_sb, i_width, ew_widths, j_order=None,
               half_cb=None):
    """Masked-softmax attention: out_sb[d, i] = sum_j attn[i,j] Wh[j,d].
    adjt_ap: DRAM AP [N, i_width] (adj^T slice, fp16). After each ew_width
    i-half completes (including normalization), half_cb(ih) is invoked so the
    caller can pipeline downstream work (ELU, h@Wo, collectives) against the
    next half's attention loop."""
    if j_order is None:
        j_order = list(range(NT))
    if isinstance(ew_widths, int):
        ew_widths = [ew_widths]
    assert sum(ew_widths) == i_width
    psum, work, small, dram = (pools["psum"], pools["work"], pools["small"],
                               pools["dram"])
    ones1 = _CACHED["ones1"]

    i0 = 0
    for ih, ew_width in enumerate(ew_widths):
        n_chunk = ew_width // 512
        accs = [psum.tile([D + 1, 512], F32, tag="bank", bufs=8,
                          name=f"acc_{tag}_{ih}_{q}") for q in range(n_chunk)]
        for jn, j in enumerate(j_order):
            u_t = work.tile([P, ew_width], dt_fast, tag="u", bufs=8,
                            name=f"u_{tag}_{ih}_{jn}")
            nc.scalar.activation(u_t[:], f2_bc[:, i0:i0 + ew_width],
                                 mybir.ActivationFunctionType.Identity,
                                 scale=r_sb[:, j:j + 1])
            nc.vector.tensor_max(u_t[:], u_t[:], f1_bc[:, i0:i0 + ew_width])
            adj_t = work.tile([P, ew_width], F16, tag="adj", bufs=10,
                              name=f"adj_{tag}_{ih}_{jn}")
            nc.sync.dma_start(out=adj_t[:],
                              in_=adjt_ap[j * P:(j + 1) * P,
                                          i0:i0 + ew_width])
            nc.vector.tensor_mul(adj_t[:], u_t[:], adj_t[:])
            for q in range(n_chunk):
                nc.tensor.matmul(accs[q][:], whs_tiles[j][:],
                                 adj_t[:, q * 512:(q + 1) * 512],
                                 start=(jn == 0), stop=(jn == NT - 1))

        # ---- per-half normalization tail ----
        # Broadcast the PSUM denominator row (partition D) down D partitions
        # with a base-64 ones-matmul, then reciprocal as exp(-ln(x)) on ACT
        # (Ln/Exp share one table set) — no cross-partition DMAs needed.
        ones64t = _CACHED["ones64t"]
        for q in range(n_chunk):
            num_sb = work.tile([D + 1, 512], F32, tag="num", bufs=4,
                               name=f"num_{tag}_{ih}_{q}")
            nc.scalar.activation(num_sb[:], accs[q][:],
                                 mybir.ActivationFunctionType.Copy)
            den_bc = psum.tile([D, 512], F32, tag="bank", bufs=8,
                               name=f"denbc_{tag}_{ih}_{q}")
            nc.tensor.matmul(den_bc[:], ones64t[D:D + 1, :],
                             num_sb[D:D + 1, :], start=True, stop=True)
            lnb = work.tile([D, 512], F32, tag="lnb", bufs=2,
                            name=f"lnb_{tag}_{ih}_{q}")
            nc.scalar.activation(lnb[:], den_bc[:],
                                 mybir.ActivationFunctionType.Ln)
            recb = work.tile([D, 512], F32, tag="recb", bufs=2,
                             name=f"recb_{tag}_{ih}_{q}")
            nc.scalar.activation(recb[:], lnb[:],
                                 mybir.ActivationFunctionType.Exp, scale=-1.0)
            nc.vector.tensor_mul(out_sb[:, i0 + q * 512:i0 + (q + 1) * 512],
                                 num_sb[0:D, :], recb[:])
        if half_cb is not None:
            half_cb(ih)
        i0 += ew_width


def _elu(nc, pools, tag, src_ap, dst_ap, width, ew=512):
    """dst = elu(src) elementwise on [D, width] fp32 tiles."""
    work = pools["work"]
    for s in range(width // ew):
        sl = slice(s * ew, (s + 1) * ew)
        t_min = work.tile([D, ew], F32, tag="elu_min", bufs=2,
                          name=f"elmin_{tag}_{s}")
        nc.vector.tensor_scalar_min(t_min[:], src_ap[:, sl], 0.0)
        t_exp = work.tile([D, ew], F32, tag="elu_exp", bufs=2,
                          name=f"elexp_{tag}_{s}")
        nc.scalar.activation(t_exp[:], t_min[:],
                             mybir.ActivationFunctionType.Exp)
        t_lin = work.tile([D, ew], F32, tag="elu_lin", bufs=2,
                          name=f"ellin_{tag}_{s}")
        nc.vector.tensor_scalar(t_lin[:], src_ap[:, sl], 0.0, -1.0,
                                mybir.AluOpType.max, mybir.AluOpType.add)
        nc.vector.tensor_add(dst_ap[:, sl], t_exp[:], t_lin[:])


def build_kernel(dt_fast=F16, repeat=1):
    nc = bacc.Bacc("TRN2", num_devices=N_CORES)

    xT = nc.dram_tensor("xT", [F, N], F16, kind="ExternalInput")
    adjT = nc.dram_tensor("adjT", [N, N], F16, kind="ExternalInput")
    adjT2 = nc.dram_tensor("adjT2", [N, SLICE], F16, kind="ExternalInput")
    Wext = nc.dram_tensor("Wext", [F, D + 2], F16, kind="ExternalInput")
    Woext = nc.dram_tensor("Woext", [D, D + 2], F32, kind="ExternalInput")
    outT = nc.dram_tensor("outT", [D, SLICE], F32, kind="ExternalOutput")

    with ExitStack() as ctx:
        tc = ctx.enter_context(tile.TileContext(nc))
        psum = ctx.enter_context(tc.tile_pool(name="psum", bufs=1, space="PSUM"))
        persist = ctx.enter_context(tc.tile_pool(name="persist", bufs=1))
        work = ctx.enter_context(tc.tile_pool(name="work", bufs=1))
        small = ctx.enter_context(tc.tile_pool(name="small", bufs=1))
        dram = ctx.enter_context(tc.tile_pool(name="dram", bufs=1, space="DRAM"))
        pools = {"psum": psum, "persist": persist, "work": work,
                 "small": small, "dram": dram}

        ident = persist.tile([P, P], F32, tag="ident")
        make_identity(nc, ident[:])
        ones1 = persist.tile([1, P], F32, tag="ones1")
        nc.vector.memset(ones1[:], 1.0)
        ones16 = persist.tile([1, P], F16, tag="ones16")
        nc.vector.memset(ones16[:], 1.0)
        _CACHED.clear()
        _CACHED["ones1"] = ones1
        _CACHED["ident"] = ident
        _CACHED["ones16"] = ones16
        bias_s = persist.tile([P, 1], F32, tag="bias_s")
        nc.vector.memset(bias_s[:], -C_SRC)
        bias_d = persist.tile([P, 1], F32, tag="bias_d")
        nc.vector.memset(bias_d[:], -C_DST)
        _CACHED["bias_s"] = bias_s
        _CACHED["bias_d"] = bias_d
        ones64t = persist.tile([D + 1, D], F32, tag="ones64t")
        nc.vector.memset(ones64t[:], 1.0)
        _CACHED["ones64t"] = ones64t

        def emit_body():
            _emit_gat(nc, pools, dt_fast, xT, adjT, adjT2, Wext, Woext, outT)

        for _rep in range(repeat):
            emit_body()

    nc.compile()
    return nc


def _emit_gat(nc, pools, dt_fast, xT, adjT, adjT2, Wext, Woext, outT):
    psum, persist, work, small, dram = (pools["psum"], pools["persist"],
                                        pools["work"], pools["small"],
                                        pools["dram"])
    if True:
        # ---- phase 1: Wh = x @ W_ext (fp16 in, fp32 accum), weight prep ----
        wext_sb = []
        for kf in range(NKF):
            t = small.tile([P, D + 2], F16, tag=f"wext{kf}", name=f"wext_{kf}")
            nc.sync.dma_start(out=t[:], in_=Wext[kf * P:(kf + 1) * P, :])
            wext_sb.append(t)
        xt_sb = []
        for kf in range(NKF):
            t = work.tile([P, N], F16, tag=f"xt{kf}", name=f"xt_{kf}")
            nc.sync.dma_start(out=t[:], in_=xT[kf * P:(kf + 1) * P, :])
            xt_sb.append(t)

        def l1_src(t):
            wh_ps = psum.tile([P, D + 2], F32, tag="bank", bufs=8,
                              name=f"whps_{t}")
            for kf in range(NKF):
                nc.tensor.matmul(wh_ps[:], xt_sb[kf][:, t * P:(t + 1) * P],
                                 wext_sb[kf][:], start=(kf == 0),
                                 stop=(kf == NKF - 1))
            return wh_ps

        whs1, r1_sb = _weight_prep(nc, pools, "l1", dt_fast, l1_src)

        # f_src row directly: fs_row[i] = sum_f wa_src[f] * xT[f,i] via M=1
        # matmuls (fp16, same precision as the Wh path), no transpose/bounce
        fs_row = work.tile([1, N], F32, tag="bigrow", name="fs_row")
        for sl in range(8):
            fr_ps = psum.tile([1, 512], F32, tag="bank", bufs=8,
                              name=f"frps_{sl}")
            for kf in range(NKF):
                nc.tensor.matmul(fr_ps[:], wext_sb[kf][:, D:D + 1],
                                 xt_sb[kf][:, sl * 512:(sl + 1) * 512],
                                 start=(kf == 0), stop=(kf == NKF - 1))
            nc.scalar.activation(fs_row[0:1, sl * 512:(sl + 1) * 512],
                                 fr_ps[:], mybir.ActivationFunctionType.Copy)
        f1_bc, f2_bc = _bcast_exp_rows(nc, pools, "l1", dt_fast, fs_row, N)

        # ---- phases 2+3 pipelined: layer-1 attention in two uneven
        # i-pieces (3072 + 1024); after each piece: ELU, h @ Wo_ext and its
        # own ReduceScatter+AllGather, so piece A's exchange and most of the
        # serial tail hide under piece B's attention loop. Core c's layer-2
        # rows are [c*384,(c+1)*384) of piece A plus [c*128,(c+1)*128) of B.
        HALF_COLS = [SPLIT_A, N - SPLIT_A]
        HALF_TILES = [SPLIT_A // P, (N - SPLIT_A) // P]
        RS_ROWS = [hc // N_CORES for hc in HALF_COLS]      # 384, 128
        o1_sb = persist.tile([D, N], F32, tag="o1")
        woext_sb = persist.tile([D, D + 2], F32, tag="woext")
        nc.sync.dma_start(out=woext_sb[:], in_=Woext[:])
        cc_in = [dram.tile([HALF_COLS[h], D + 2], F16, tag=f"cc_in{h}",
                           name=f"cc_in{h}") for h in range(2)]
        cc_rs = [dram.tile([RS_ROWS[h], D + 2], F16, tag=f"cc_rs{h}",
                           name=f"cc_rs{h}") for h in range(2)]
        cc_full = [dram.tile([HALF_COLS[h], D + 2], F16, tag=f"cc_full{h}",
                             addr_space="Shared", name=f"cc_full{h}")
                   for h in range(2)]

        def l1_half_done(h):
            lo = sum(HALF_COLS[:h])
            t0 = sum(HALF_TILES[:h])
            _elu(nc, pools, f"l1h{h}", o1_sb[:, lo:lo + HALF_COLS[h]],
                 o1_sb[:, lo:lo + HALF_COLS[h]], HALF_COLS[h])
            for tt in range(HALF_TILES[h]):
                t = t0 + tt
                p2_ps = psum.tile([P, D + 2], F32, tag="bank", bufs=8,
                                  name=f"p2ps_{t}")
                nc.tensor.matmul(p2_ps[:], o1_sb[:, t * P:(t + 1) * P],
                                 woext_sb[:], start=True, stop=True)
                p2_sb = work.tile([P, D + 2], F16, tag="stage66", bufs=4,
                                  name=f"p2sb_{t}")
                nc.scalar.activation(p2_sb[:], p2_ps[:],
                                     mybir.ActivationFunctionType.Copy)
                nc.sync.dma_start(out=cc_in[h][tt * P:(tt + 1) * P, :],
                                  in_=p2_sb[:])
            nc.gpsimd.collective_compute(
                "ReduceScatter", mybir.AluOpType.add,
                ins=[cc_in[h][:]], outs=[cc_rs[h][:]],
                replica_groups=[list(range(N_CORES))])
            nc.gpsimd.collective_compute(
                "AllGather", mybir.AluOpType.bypass,
                ins=[cc_rs[h][:]], outs=[cc_full[h][:]],
                replica_groups=[list(range(N_CORES))])

        _attention(nc, pools, "l1", dt_fast, whs1, r1_sb, f1_bc, f2_bc,
                   adjT[:], o1_sb, N, HALF_COLS, half_cb=l1_half_done)

        # ---- phase 4: layer-2 prep (piece-A tiles arrive first) ----
        def l2_src(t):
            h = 0 if t < HALF_TILES[0] else 1
            tt = t - (0 if h == 0 else HALF_TILES[0])
            s = work.tile([P, D + 2], F16, tag="ccsb", bufs=4,
                          name=f"ccsb_{t}")
            nc.sync.dma_start(out=s[:],
                              in_=cc_full[h][tt * P:(tt + 1) * P, :])
            return s

        whs2, r2_sb = _weight_prep(nc, pools, "l2", dt_fast, l2_src)

        fs2_row = small.tile([1, SLICE], F16, tag="fs2_row")
        off = 0
        for h in range(2):
            nc.sync.dma_start(
                out=fs2_row[0:1, off:off + RS_ROWS[h]],
                in_=cc_rs[h][:, D:D + 1].rearrange("n one -> one n"))
            off += RS_ROWS[h]
        f1_bc2, f2_bc2 = _bcast_exp_rows(nc, pools, "l2", dt_fast, fs2_row,
                                         SLICE)

        # ---- layer-2 attention on this core's row slices, ELU, store ----
        o2_sb = persist.tile([D, SLICE], F32, tag="o2")
        _attention(nc, pools, "l2", dt_fast, whs2, r2_sb, f1_bc2, f2_bc2,
                   adjT2[:], o2_sb, SLICE, [512])
        fin = persist.tile([D, SLICE], F32, tag="fin")
        _elu(nc, pools, "l2", o2_sb, fin, SLICE)
        nc.sync.dma_start(out=outT[:], in_=fin[:])


# ---------------------------------------------------------------------------
# host-side driver
# ---------------------------------------------------------------------------

def _prep_inputs(x, adj, W, a, Wo, ao):
    xT = np.ascontiguousarray(x.T.astype(np.float16))
    adjT = np.ascontiguousarray(adj.T.astype(np.float16))
    in_maps = []
    for c in range(N_CORES):
        a_src, a_dst = a[c, :D], a[c, D:]
        wext = np.concatenate(
            [W[c], (W[c] @ a_src)[:, None], (W[c] @ a_dst)[:, None]],
            axis=1).astype(np.float16)
        Wo_h = Wo[c * D:(c + 1) * D]
        woext = np.concatenate(
            [Wo_h, (Wo_h @ ao[:D])[:, None], (Wo_h @ ao[D:])[:, None]],
            axis=1).astype(np.float32)
        adjt2 = np.concatenate(
            [adjT[:, c * RS_A:(c + 1) * RS_A],
             adjT[:, SPLIT_A + c * RS_B:SPLIT_A + (c + 1) * RS_B]], axis=1)
        in_maps.append({
            "xT": xT,
            "adjT": adjT,
            "adjT2": np.ascontiguousarray(adjt2),
            "Wext": wext,
            "Woext": woext,
        })
    return in_maps


def kernel(x, adj, W, a, Wo, ao, cfg):
    x = np.asarray(x, np.float32)
    adj = np.asarray(adj, np.float32)
    W = np.asarray(W, np.float32)
    a = np.asarray(a, np.float32)
    Wo = np.asarray(Wo, np.float32)
    ao = np.asarray(ao, np.float32)

    in_maps = _prep_inputs(x, adj, W, a, Wo, ao)
    if _CACHED.get("nc") is None:
        nc = build_kernel()
        _CACHED["nc"] = nc
    res = run_bass_kernel_spmd(_CACHED["nc"], in_maps,
                               core_ids=list(range(N_CORES)))
    out = np.empty((N, D), np.float32)
    for c in range(N_CORES):
        oT = res.results[c]["outT"]
        out[c * RS_A:(c + 1) * RS_A, :] = oT[:, 0:RS_A].T
        out[SPLIT_A + c * RS_B:SPLIT_A + (c + 1) * RS_B, :] = oT[:, RS_A:].T
    return out


if __name__ == "__main__":
    import reference as ref_mod
    inputs = {k: np.asarray(v) for k, v in ref_mod.setup_inputs().items()}
    expected = np.asarray(ref_mod.reference(**ref_mod.setup_inputs()))
    got = kernel(**inputs)
    err = np.abs(got - expected).max() / np.abs(expected).max()
    print("rel err:", err)
